# revision 33
# baseline (speedup 1.0000x reference)
"""Trainium2 Bass kernel for nn_Attention_85813446574600.

Reference computes:
    s_x = x @ W[:F] + b            # [B,T,1]
    s_c = context @ W[F:]          # [C,1]
    scores = s_x + s_c             # [B,T,C,1]
    att = softmax(scores, axis=-1) # softmax over a SIZE-1 axis -> exactly 1.0
    out = einsum('btc,btf->bcf', att, x)

Since softmax over the last (size-1) axis is identically 1.0 for any finite
scores, the output is exactly out[b,c,f] = sum_t x[b,t,f], independent of c
(and of context/W/b entirely).

The problem is pure data movement (read x, time-reduce, broadcast over C).
The rel-err budget (2e-2) is spent on fp16 I/O to halve HBM traffic in both
directions: the host casts x to fp16, the device accumulates in f32 (PE
matmul with an all-ones stationary matrix sums over partitions AND
broadcasts to all 128 output partitions; PSUM accumulation folds the 4
T-rows-per-partition column slices), and the [B_LOC, C, F] output is
written as fp16 and upcast on the host.  Measured end-to-end error is
~1e-3 vs the 2e-2 gate.

Per core (batch-sharded 32/8 = 4 batches), raw Bass (no Tile framework).
Measured design rules this schedule is built on:

  - DMA descriptor size sets ring throughput (8KB -> ~420 GB/s, 4KB ->
    ~310, 2KB -> ~145 per ring), so input loads are whole batches
    (partition p <- 4 consecutive T rows = 4KB contiguous).
  - Input is striped unevenly over the two HWDGE rings (qSP: b0,b1,b2;
    qAct: b3) so batch arrivals spread out instead of bunching, and the
    output traffic never rides the input ring until the stream is done.
  - One wide fp16 DVE add per batch folds the 4 T-rows-per-partition to
    2, halving the PE column stream; ones[128,128] @ folded-halves
    accumulated in one PSUM bank sums over partitions AND broadcasts to
    all 128 output rows.  PSUM->SBUF fp16 casts run on DVE (b0,b1,b2)
    and ACT (b3, which reads PSUM natively) in arrival order.
  - b0/b3 outputs are single-dispatch [256,F] slab DMAs whose source AP
    re-reads ots[b] via a stride-0 dim; b1 rides the third DMA path
    (GPSIMD/SWDGE); the tail batch b2 goes as two half-slabs dispatched
    in parallel from SYNC and ACT.
  - NO completion waits on output DMAs: the runtime's per-execution
    teardown (each engine resets ~51 semaphores starting ~1us after its
    own last instruction; Tensor 115ns/reset = 6.1us, Scalar 90ns,
    Sync 45ns) is far longer than the in-flight outputs need to drain.
    The teardown rate is fixed (unaffected by clock state), so each
    engine's user work just ends as early as possible.
  - 24 packed dummy matmuls before the first real matmul ramp the PE
    clock (real [128,512] matmuls: ~585ns warmed vs ~730ns cold); more
    dummies would trip the HAM duty-cycle throttle into a forced
    half-rate window.
  - The bass init/Block-exit all-engine barriers are patched out: the
    runtime's own end-of-execution protocol provides the sync.
"""

import sys

for _p in ("/opt/trn_rl_repo",):
    if _p not in sys.path:
        sys.path.insert(0, _p)

from contextlib import ExitStack

import numpy as np

import concourse.bass as bass
import concourse.mybir as mybir
from concourse.bass_utils import run_bass_kernel_spmd

# Problem shapes (hardcoded per harness contract)
B, T, C, F = 32, 512, 256, 512
N_CORES = 8
B_LOC = B // N_CORES  # 4 batches per core
P = 128               # SBUF/PSUM partitions
TT = T // P           # 4 T-rows folded into each partition
DT = mybir.dt.float16
DT_NP = np.float16
DTACC = mybir.dt.float32

_NC_CACHE = {}


def _build_nc():
    # Bass.__init__ ends with const-AP memsets plus an all-engine barrier,
    # and BassBlock.__exit__ emits another all-engine barrier; neither is
    # needed here (every cross-engine dependency is explicitly
    # semaphore-gated, and the Neuron runtime runs its own all-engine
    # barrier before its per-execution teardown), so patch the barrier out
    # for the whole build: the first input DMA issues ~0.4us sooner and the
    # tail loses ~0.7us of duplicate barrier.
    _orig_barrier = bass.Bass.all_engine_barrier
    bass.Bass.all_engine_barrier = lambda self, sem_only=False: None
    try:
        nc = bass.Bass("TRN2", target_bir_lowering=False)
        x = nc.dram_tensor("x", [B_LOC, T, F], DT, kind="ExternalInput").ap()
        out = nc.dram_tensor("out", [B_LOC, C, F], DT, kind="ExternalOutput").ap()

        with ExitStack() as ctx:
            ec = ctx.enter_context
            ones = ec(nc.sbuf_tensor("ones", [P, P], DT)).ap()
            xts = [
                ec(nc.sbuf_tensor(f"xt{b}", [P, TT * F], DT)).ap()
                for b in range(B_LOC)
            ]
            halves = [
                ec(nc.sbuf_tensor(f"hv{b}", [P, 2 * F], DT)).ap()
                for b in range(B_LOC)
            ]
            ots = [
                ec(nc.sbuf_tensor(f"ot{b}", [P, F], DT)).ap() for b in range(B_LOC)
            ]
            accs = [
                ec(nc.psum_tensor(f"acc{b}", [P, F], DTACC)).ap()
                for b in range(B_LOC)
            ]
            warm_ps = ec(nc.psum_tensor("warm_ps", [P, P], DTACC)).ap()

            in_sems = [ec(nc.semaphore(f"in_sem{b}")) for b in range(B_LOC)]
            vec_sem = ec(nc.semaphore("vec_sem"))
            av_sems = [ec(nc.semaphore(f"av_sem{b}")) for b in range(B_LOC)]
            pe_sems = [ec(nc.semaphore(f"pe_sem{b}")) for b in range(B_LOC)]
            cp_sems = [ec(nc.semaphore(f"cp_sem{b}")) for b in range(B_LOC)]
            osem_sp = ec(nc.semaphore("osem_sp"))
            osem_act = ec(nc.semaphore("osem_act"))

            block = ec(nc.Block())

            def out_half(eng, b, h, sem):
                # one 128-row half of out[b]; all 256 rows are identical
                return eng.dma_start(
                    out[b, h * P : (h + 1) * P, :], ots[b]
                ).then_inc(sem, 16)

            def in_load(eng, b):
                # whole batch b: partition p <- 4 consecutive T rows
                # (4KB contiguous per partition; descriptor size sets the
                # ring rate: 4KB descs run ~2x faster than 2KB)
                src = x[b].rearrange("(p l) f -> p l f", p=P)
                return eng.dma_start(
                    xts[b].rearrange("p (l f) -> p l f", l=TT), src
                ).then_inc(in_sems[b], 16)

            # Input striping is deliberately UNEVEN: qSP carries b0,b1,b2
            # and qAct only b3.  The batch arrivals then spread out
            # (~2 rings concurrently for the first wave, then qSP alone)
            # instead of both rings delivering their second batch at the
            # same instant -- which would bunch all the tail work
            # (adds/matmuls/casts/output issues) into one serialized burst.
            # qAct is drained early and becomes the output ring from ~4us.
            #
            # NO completion waits on the output DMAs: the runtime's
            # per-execution teardown (~7us of semaphore resets after the
            # engines' last instructions) is far longer than the ~2.5us the
            # in-flight output DMAs need to drain, so the results are in
            # DRAM long before the execution completes.  Dropping the
            # receipt waits takes ~2.4us off the critical path.
            def out_slab(eng, b, sem):
                # whole [256, F] slab in ONE dispatch: the DMA source AP
                # re-reads ots[b] with a stride-0 middle dim (rows p and
                # 128+p of the slab both come from partition p)
                dst = out[b].rearrange("(h p) f -> p h f", h=2)
                src = ots[b].unsqueeze(1).broadcast_to([P, 2, F])
                return eng.dma_start(dst, src).then_inc(sem, 16)

            # qSP carries ONLY input until the stream is done (output data
            # on the same ring was measured to delay the last input batch
            # by ~1.5us), then the single tail half-slab.
            # Engine-assignment principle: the runtime teardown resets ~51
            # semaphores per engine starting ~1us after EACH ENGINE's OWN
            # last instruction, with per-engine reset rates (Tensor 115ns,
            # Scalar 90ns, Vector 68ns, GpSimd 54ns, Sync 45ns each).  The
            # kernel end is max(engine_end + its reset chain), so the
            # engines with the longest chains (Tensor 6.1us, Scalar 4.6us)
            # must finish their user work FIRST.
            @block.sync
            def _(sync):
                in_load(sync, 0)
                in_load(sync, 1)
                in_load(sync, 2)
                sync.wait_ge(cp_sems[2], 1)
                out_half(sync, 2, 1, osem_sp)

            @block.scalar
            def _(scalar):
                in_load(scalar, 3)
                scalar.wait_ge(cp_sems[0], 1)
                out_slab(scalar, 0, osem_act)
                scalar.wait_ge(pe_sems[3], 1)
                nc.scalar.copy(ots[3], accs[3]).then_inc(cp_sems[3], 1)
                scalar.wait_ge(cp_sems[3], 1)
                out_slab(scalar, 3, osem_act)
                scalar.wait_ge(cp_sems[2], 1)
                out_half(scalar, 2, 0, osem_act)

            # GPSIMD: b1's output halves on the third DMA path (SWDGE /
            # qPool); the Pool engine cannot read PSUM, so b1's cast stays
            # on the DVE
            @block.gpsimd
            def _(gpsimd):
                gpsimd.wait_ge(cp_sems[1], 1)
                nc.gpsimd.dma_start(out[1, 0:P, :], ots[1]).then_inc(
                    osem_sp, 16
                )
                nc.gpsimd.dma_start(out[1, P:C, :], ots[1]).then_inc(
                    osem_sp, 16
                )

            # DVE: one wide fp16 add per batch folds the 4 T-rows-per-
            # partition down to 2 (halving the PE column stream), plus the
            # PSUM->SBUF fp16 casts, interleaved in arrival order.
            @block.vector
            def _(vector):
                def add(b):
                    vector.wait_ge(in_sems[b], 16)
                    nc.vector.tensor_add(
                        halves[b], xts[b][:, 0 : 2 * F], xts[b][:, 2 * F : 4 * F]
                    ).then_inc(av_sems[b], 1)

                def cast(b):
                    vector.wait_ge(pe_sems[b], 1)
                    nc.vector.tensor_copy(ots[b], accs[b]).then_inc(
                        cp_sems[b], 1
                    )

                nc.vector.memset(ones, 1.0).then_inc(vec_sem, 1)
                add(0)
                add(3)
                cast(0)
                add(1)
                add(2)
                cast(1)
                cast(2)

            # PE sequencer must END as early as possible (longest teardown
            # chain); its reset sweep starts from its last SEQUENCER
            # instruction while the array drains asynchronously.  The
            # early dummy burst ramps the clock so the real matmuls run
            # ~585ns instead of ~730ns; NO trailing dummies (they would
            # push the sequencer end, and the teardown rate is fixed).
            @block.tensor
            def _(tensor):
                tensor.wait_ge(vec_sem, 1)
                for _ in range(24):
                    nc.tensor.matmul(warm_ps, ones, ones, start=True, stop=True)
                for b in (0, 3, 1, 2):
                    tensor.wait_ge(av_sems[b], 1)
                    nc.tensor.matmul(
                        accs[b], ones, halves[b][:, 0:F], start=True, stop=False
                    )
                    nc.tensor.matmul(
                        accs[b], ones, halves[b][:, F : 2 * F], start=False, stop=True
                    ).then_inc(pe_sems[b], 1)
    finally:
        bass.Bass.all_engine_barrier = _orig_barrier

    return nc


def _get_nc():
    if "nc" not in _NC_CACHE:
        _NC_CACHE["nc"] = _build_nc()
    return _NC_CACHE["nc"]


def make_in_maps(x):
    """Shard + fp16-quantize the full [B,T,F] input for the 8 cores."""
    xh = np.ascontiguousarray(np.asarray(x), dtype=DT_NP)
    return [{"x": xh[i * B_LOC : (i + 1) * B_LOC]} for i in range(N_CORES)]


def kernel(x, context=None, W=None, b=None, **_unused):
    """Full inputs in, full output out. context/W/b provably do not affect
    the output (softmax over a size-1 axis is identically 1)."""
    x = np.asarray(x)
    assert x.shape == (B, T, F), x.shape

    nc = _get_nc()
    res = run_bass_kernel_spmd(nc, make_in_maps(x), core_ids=list(range(N_CORES)))
    out = np.concatenate([r["out"] for r in res.results], axis=0)
    return out.astype(np.float32)


# revision 37
# speedup vs baseline: 1.0022x; 1.0022x over previous
"""Trainium2 Bass kernel for nn_Attention_85813446574600.

Reference computes:
    s_x = x @ W[:F] + b            # [B,T,1]
    s_c = context @ W[F:]          # [C,1]
    scores = s_x + s_c             # [B,T,C,1]
    att = softmax(scores, axis=-1) # softmax over a SIZE-1 axis -> exactly 1.0
    out = einsum('btc,btf->bcf', att, x)

Since softmax over the last (size-1) axis is identically 1.0 for any finite
scores, the output is exactly out[b,c,f] = sum_t x[b,t,f], independent of c
(and of context/W/b entirely).

The problem is pure data movement (read x, time-reduce, broadcast over C).
The rel-err budget (2e-2) is spent on fp16 I/O to halve HBM traffic in both
directions: the host casts x to fp16, the device accumulates in f32 (PE
matmul with an all-ones stationary matrix sums over partitions AND
broadcasts to all 128 output partitions; PSUM accumulation folds the 4
T-rows-per-partition column slices), and the [B_LOC, C, F] output is
written as fp16 and upcast on the host.  Measured end-to-end error is
~1e-3 vs the 2e-2 gate.

Per core (batch-sharded 32/8 = 4 batches), raw Bass (no Tile framework).
Measured design rules this schedule is built on:

  - DMA descriptor size sets ring throughput (8KB -> ~420 GB/s, 4KB ->
    ~310, 2KB -> ~145 per ring), so input loads are whole batches
    (partition p <- 4 consecutive T rows = 4KB contiguous).
  - Input is striped unevenly over the two HWDGE rings (qSP: b0,b1,b2;
    qAct: b3) so batch arrivals spread out instead of bunching, and the
    output traffic never rides the input ring until the stream is done.
  - One wide fp16 DVE add per batch folds the 4 T-rows-per-partition to
    2, halving the PE column stream; ones[128,128] @ folded-halves
    accumulated in one PSUM bank sums over partitions AND broadcasts to
    all 128 output rows.  PSUM->SBUF fp16 casts run on DVE (b0,b1,b2)
    and ACT (b3, which reads PSUM natively) in arrival order.
  - b0/b3 outputs are single-dispatch [256,F] slab DMAs whose source AP
    re-reads ots[b] via a stride-0 dim; b1 rides the third DMA path
    (GPSIMD/SWDGE); the tail batch b2 goes as two half-slabs dispatched
    in parallel from SYNC and ACT.
  - NO completion waits on output DMAs: the runtime's per-execution
    teardown (each engine resets ~51 semaphores starting ~1us after its
    own last instruction; Tensor 115ns/reset = 6.1us, Scalar 90ns,
    Sync 45ns) is far longer than the in-flight outputs need to drain.
    The teardown rate is fixed (unaffected by clock state), so each
    engine's user work just ends as early as possible.
  - 24 packed dummy matmuls before the first real matmul ramp the PE
    clock (real [128,512] matmuls: ~585ns warmed vs ~730ns cold); more
    dummies would trip the HAM duty-cycle throttle into a forced
    half-rate window.
  - The bass init/Block-exit all-engine barriers are patched out: the
    runtime's own end-of-execution protocol provides the sync.
"""

import sys

for _p in ("/opt/trn_rl_repo",):
    if _p not in sys.path:
        sys.path.insert(0, _p)

from contextlib import ExitStack

import numpy as np

import concourse.bass as bass
import concourse.mybir as mybir
from concourse.bass_utils import run_bass_kernel_spmd

# Problem shapes (hardcoded per harness contract)
B, T, C, F = 32, 512, 256, 512
N_CORES = 8
B_LOC = B // N_CORES  # 4 batches per core
P = 128               # SBUF/PSUM partitions
TT = T // P           # 4 T-rows folded into each partition
DT = mybir.dt.float16
DT_NP = np.float16
DTACC = mybir.dt.float32

_NC_CACHE = {}


def _build_nc():
    # Bass.__init__ ends with const-AP memsets plus an all-engine barrier,
    # and BassBlock.__exit__ emits another all-engine barrier; neither is
    # needed here (every cross-engine dependency is explicitly
    # semaphore-gated, and the Neuron runtime runs its own all-engine
    # barrier before its per-execution teardown), so patch the barrier out
    # for the whole build: the first input DMA issues ~0.4us sooner and the
    # tail loses ~0.7us of duplicate barrier.
    _orig_barrier = bass.Bass.all_engine_barrier
    bass.Bass.all_engine_barrier = lambda self, sem_only=False: None
    try:
        nc = bass.Bass("TRN2", target_bir_lowering=False)
        x = nc.dram_tensor("x", [B_LOC, T, F], DT, kind="ExternalInput").ap()
        out = nc.dram_tensor("out", [B_LOC, C, F], DT, kind="ExternalOutput").ap()

        with ExitStack() as ctx:
            ec = ctx.enter_context
            ones = ec(nc.sbuf_tensor("ones", [P, P], DT)).ap()
            xts = [
                ec(nc.sbuf_tensor(f"xt{b}", [P, TT * F], DT)).ap()
                for b in range(B_LOC)
            ]
            halves = [
                ec(nc.sbuf_tensor(f"hv{b}", [P, 2 * F], DT)).ap()
                for b in range(B_LOC)
            ]
            ots = [
                ec(nc.sbuf_tensor(f"ot{b}", [P, F], DT)).ap() for b in range(B_LOC)
            ]
            accs = [
                ec(nc.psum_tensor(f"acc{b}", [P, F], DTACC)).ap()
                for b in range(B_LOC)
            ]
            warm_ps = ec(nc.psum_tensor("warm_ps", [P, P], DTACC)).ap()

            in_sems = [ec(nc.semaphore(f"in_sem{b}")) for b in range(B_LOC)]
            in2b_sem = ec(nc.semaphore("in2b_sem"))
            vec_sem = ec(nc.semaphore("vec_sem"))
            av_sems = [ec(nc.semaphore(f"av_sem{b}")) for b in range(B_LOC)]
            av2b_sem = ec(nc.semaphore("av2b_sem"))
            pe_sems = [ec(nc.semaphore(f"pe_sem{b}")) for b in range(B_LOC)]
            cp_sems = [ec(nc.semaphore(f"cp_sem{b}")) for b in range(B_LOC)]
            osem_sp = ec(nc.semaphore("osem_sp"))
            osem_act = ec(nc.semaphore("osem_act"))

            block = ec(nc.Block())

            def out_half(eng, b, h, sem):
                # one 128-row half of out[b]; all 256 rows are identical
                return eng.dma_start(
                    out[b, h * P : (h + 1) * P, :], ots[b]
                ).then_inc(sem, 16)

            def in_load(eng, b):
                # whole batch b: partition p <- 4 consecutive T rows
                # (4KB contiguous per partition; descriptor size sets the
                # ring rate: 4KB descs run ~2x faster than 2KB)
                src = x[b].rearrange("(p l) f -> p l f", p=P)
                return eng.dma_start(
                    xts[b].rearrange("p (l f) -> p l f", l=TT), src
                ).then_inc(in_sems[b], 16)

            # Input striping is deliberately UNEVEN: qSP carries b0,b1,b2
            # and qAct only b3.  The batch arrivals then spread out
            # (~2 rings concurrently for the first wave, then qSP alone)
            # instead of both rings delivering their second batch at the
            # same instant -- which would bunch all the tail work
            # (adds/matmuls/casts/output issues) into one serialized burst.
            # qAct is drained early and becomes the output ring from ~4us.
            #
            # NO completion waits on the output DMAs: the runtime's
            # per-execution teardown (~7us of semaphore resets after the
            # engines' last instructions) is far longer than the ~2.5us the
            # in-flight output DMAs need to drain, so the results are in
            # DRAM long before the execution completes.  Dropping the
            # receipt waits takes ~2.4us off the critical path.
            def out_slab(eng, b, sem):
                # whole [256, F] slab in ONE dispatch: the DMA source AP
                # re-reads ots[b] with a stride-0 middle dim (rows p and
                # 128+p of the slab both come from partition p)
                dst = out[b].rearrange("(h p) f -> p h f", h=2)
                src = ots[b].unsqueeze(1).broadcast_to([P, 2, F])
                return eng.dma_start(dst, src).then_inc(sem, 16)

            # qSP carries ONLY input until the stream is done (output data
            # on the same ring was measured to delay the last input batch
            # by ~1.5us), then the single tail half-slab.
            def half_load(eng, h, sem):
                # one half of batch 2: T rows h*256..h*256+255, partition
                # p <- 2 consecutive rows (2KB contiguous per partition)
                src = x[2].rearrange("(h p l) f -> h p l f", h=2, p=P)
                return eng.dma_start(
                    xts[2][:, h * 2 * F : (h + 1) * 2 * F].rearrange(
                        "p (l f) -> p l f", l=2
                    ),
                    src[h],
                ).then_inc(sem, 16)

            # Engine-assignment principle: the runtime teardown resets ~51
            # semaphores per engine starting ~1us after EACH ENGINE's OWN
            # last instruction, with per-engine reset rates (Tensor 115ns,
            # Scalar 90ns, Vector 68ns, GpSimd 54ns, Sync 45ns each).  The
            # kernel end is max(engine_end + its reset chain), so the
            # engines with the longest chains (Tensor 6.1us, Scalar 4.6us)
            # must finish their user work FIRST.
            #
            # The tail batch b2 streams as TWO cross-ring half-loads so
            # its fold/matmul work pipelines with the tail of the input
            # stream instead of serializing after it.
            @block.sync
            def _(sync):
                in_load(sync, 0)
                in_load(sync, 1)
                half_load(sync, 1, in2b_sem)
                sync.wait_ge(cp_sems[1], 1)
                out_slab(sync, 1, osem_sp)
                sync.wait_ge(cp_sems[2], 1)
                out_half(sync, 2, 1, osem_sp)

            @block.scalar
            def _(scalar):
                in_load(scalar, 3)
                half_load(scalar, 0, in_sems[2])
                scalar.wait_ge(pe_sems[0], 1)
                nc.scalar.copy(ots[0], accs[0]).then_inc(cp_sems[0], 1)
                scalar.wait_ge(cp_sems[0], 1)
                out_slab(scalar, 0, osem_act)
                scalar.wait_ge(pe_sems[3], 1)
                nc.scalar.copy(ots[3], accs[3]).then_inc(cp_sems[3], 1)
                scalar.wait_ge(cp_sems[3], 1)
                out_slab(scalar, 3, osem_act)

            # GPSIMD (short chain): the tail batch's other half-slab
            @block.gpsimd
            def _(gpsimd):
                gpsimd.wait_ge(cp_sems[2], 1)
                nc.gpsimd.dma_start(out[2, 0:P, :], ots[2]).then_inc(
                    osem_sp, 16
                )

            # DVE: one wide fp16 add per batch folds the 4 T-rows-per-
            # partition down to 2 (halving the PE column stream), plus
            # b1/b2's PSUM->SBUF fp16 casts (b0/b3's run on ACT),
            # interleaved in arrival order.
            @block.vector
            def _(vector):
                def add(b):
                    vector.wait_ge(in_sems[b], 16)
                    nc.vector.tensor_add(
                        halves[b], xts[b][:, 0 : 2 * F], xts[b][:, 2 * F : 4 * F]
                    ).then_inc(av_sems[b], 1)

                def cast(b):
                    vector.wait_ge(pe_sems[b], 1)
                    nc.vector.tensor_copy(ots[b], accs[b]).then_inc(
                        cp_sems[b], 1
                    )

                nc.vector.memset(ones, 1.0).then_inc(vec_sem, 1)
                add(0)
                add(3)
                add(1)
                # b2's fold in two halves, each as soon as its half lands
                vector.wait_ge(in_sems[2], 16)
                nc.vector.tensor_add(
                    halves[2][:, 0:F], xts[2][:, 0:F], xts[2][:, F : 2 * F]
                ).then_inc(av_sems[2], 1)
                vector.wait_ge(in2b_sem, 16)
                nc.vector.tensor_add(
                    halves[2][:, F : 2 * F],
                    xts[2][:, 2 * F : 3 * F],
                    xts[2][:, 3 * F : 4 * F],
                ).then_inc(av2b_sem, 1)
                cast(1)
                cast(2)

            # PE sequencer must END as early as possible (longest teardown
            # chain); its reset sweep starts from its last SEQUENCER
            # instruction while the array drains asynchronously.  The
            # early dummy burst ramps the clock so the real matmuls run
            # ~585ns instead of ~730ns; NO trailing dummies (they would
            # push the sequencer end, and the teardown rate is fixed).
            @block.tensor
            def _(tensor):
                tensor.wait_ge(vec_sem, 1)
                for _ in range(24):
                    nc.tensor.matmul(warm_ps, ones, ones, start=True, stop=True)
                for b in (0, 3, 1):
                    tensor.wait_ge(av_sems[b], 1)
                    nc.tensor.matmul(
                        accs[b], ones, halves[b][:, 0:F], start=True, stop=False
                    )
                    nc.tensor.matmul(
                        accs[b], ones, halves[b][:, F : 2 * F], start=False, stop=True
                    ).then_inc(pe_sems[b], 1)
                # b2's two matmuls gated on its two half-folds separately
                tensor.wait_ge(av_sems[2], 1)
                nc.tensor.matmul(
                    accs[2], ones, halves[2][:, 0:F], start=True, stop=False
                )
                tensor.wait_ge(av2b_sem, 1)
                nc.tensor.matmul(
                    accs[2], ones, halves[2][:, F : 2 * F], start=False, stop=True
                ).then_inc(pe_sems[2], 1)
    finally:
        bass.Bass.all_engine_barrier = _orig_barrier

    return nc


def _get_nc():
    if "nc" not in _NC_CACHE:
        _NC_CACHE["nc"] = _build_nc()
    return _NC_CACHE["nc"]


def make_in_maps(x):
    """Shard + fp16-quantize the full [B,T,F] input for the 8 cores."""
    xh = np.ascontiguousarray(np.asarray(x), dtype=DT_NP)
    return [{"x": xh[i * B_LOC : (i + 1) * B_LOC]} for i in range(N_CORES)]


def kernel(x, context=None, W=None, b=None, **_unused):
    """Full inputs in, full output out. context/W/b provably do not affect
    the output (softmax over a size-1 axis is identically 1)."""
    x = np.asarray(x)
    assert x.shape == (B, T, F), x.shape

    nc = _get_nc()
    res = run_bass_kernel_spmd(nc, make_in_maps(x), core_ids=list(range(N_CORES)))
    out = np.concatenate([r["out"] for r in res.results], axis=0)
    return out.astype(np.float32)
